# revision 5
# baseline (speedup 1.0000x reference)
"""Trainium2 Bass kernel for MimiAttention (GQA + RoPE + causal softmax).

Problem: B=2, S=2048, H=1024, NH=16 q-heads, NKV=4 kv-heads, HD=64.
Sharding: 8 cores = 2 (batch) x 4 (kv-group).  Each core computes one batch's
attention for one GQA group (4 q-heads sharing 1 kv head) and the partial
o-projection for those heads; the host sums the 4 partials per batch.

Pipeline (v2 — ACT-overlapped):
  * RoPE via the sign-permuted weight trick (W2 rows): per head the projection
    produces [q*cos ; q2*sin] on 128 partitions, khat = [k_rot;k_rot] via one
    fold matmul, so scores contract over 128 dims in one matmul per 512 cols.
  * Scores psum: 2x [128,1024] fp32 tiles (4 banks); exp runs in up-to-1024
    col batches (one ACT instruction per unit) to amortize the ACT init cost.
  * The jt loop is software-pipelined by one: scores+exp of step N+1 are
    issued before attnV of step N, so PE works while ACT runs exp.
  * attnV accumulates [i,65] slices in 3 psum banks (col 64 = softmax denom
    via a ones-column in v); normalize per (h, jt) on DVE.
  * o-projection is spread across h==3, jt>=8, borrowing the score psum pool
    slots (keeps both streams double-buffered); last 512 cols in the tail.
"""

import numpy as np
import ml_dtypes

B, S, H = 2, 2048, 1024
NH, NKV, HD = 16, 4, 64
G = NH // NKV            # 4 q-heads per kv head
THETA = 10000.0
N_CORES = 8

BF16 = ml_dtypes.bfloat16


def _build_nc():
    import concourse.mybir as mybir
    import concourse.tile as tile
    from concourse.tile import add_dep_helper
    from concourse import bacc

    f32 = mybir.dt.float32
    bf16 = mybir.dt.bfloat16

    nc = bacc.Bacc("TRN2", target_bir_lowering=False)

    xTd = nc.dram_tensor("xT", [H, S], bf16, kind="ExternalInput")
    wqkd = nc.dram_tensor("wqkT", [H, 640], bf16, kind="ExternalInput")
    wvd = nc.dram_tensor("wvT", [H, HD], bf16, kind="ExternalInput")
    csd = nc.dram_tensor("cs", [128, S], bf16, kind="ExternalInput")
    wod = nc.dram_tensor("woT", [G * HD, H], bf16, kind="ExternalInput")
    djd = nc.dram_tensor("dupJ", [128, 128], bf16, kind="ExternalInput")
    idd = nc.dram_tensor("ident", [128, 128], bf16, kind="ExternalInput")
    trid = nc.dram_tensor("trimask", [128, 128], bf16, kind="ExternalInput")
    oTd = nc.dram_tensor("oT", [H, S], bf16, kind="ExternalOutput")

    NSB = S // 512        # 4 chunks of 512
    NST = S // 128        # 16 tiles of 128
    KC = H // 128         # 8 contraction chunks
    scale = float(1.0 / np.sqrt(HD))

    with tile.TileContext(nc) as tc:
        import contextlib
        ctx = contextlib.ExitStack()
        with ctx:
            consts = ctx.enter_context(tc.tile_pool(name="consts", bufs=1))
            acts = ctx.enter_context(tc.tile_pool(name="acts", bufs=1))
            anp = ctx.enter_context(tc.tile_pool(name="attn", bufs=1))
            rcp = ctx.enter_context(tc.tile_pool(name="rcp", bufs=6))
            etp = ctx.enter_context(tc.tile_pool(name="etri", bufs=4))
            ep = ctx.enter_context(tc.tile_pool(name="exps", bufs=1))
            otp = ctx.enter_context(tc.tile_pool(name="ot", bufs=8))
            # PSUM: 2x [128,1024] score tiles (4 banks, also host o-proj
            # chunks at h==3), attnV accum (3 banks), 1 shared bank.
            pT = ctx.enter_context(
                tc.tile_pool(name="pT", bufs=2, space="PSUM"))
            pav = ctx.enter_context(
                tc.tile_pool(name="ps_av", bufs=1, space="PSUM"))
            pR7 = ctx.enter_context(
                tc.tile_pool(name="r7", bufs=1, space="PSUM"))

            # ---- input DMAs (issued from Pool seq; cheap dispatch).
            wqk_sb = consts.tile([128, KC, 640], bf16, tag="wqk")
            for kc in range(KC):
                nc.gpsimd.dma_start(
                    wqk_sb[:, kc, :], wqkd[kc * 128:(kc + 1) * 128, :])
            cs_sb = consts.tile([128, S], bf16, tag="cs")
            nc.gpsimd.dma_start(cs_sb, csd[:, :])
            dj_sb = consts.tile([128, 128], bf16, tag="dj")
            nc.gpsimd.dma_start(dj_sb, djd[:, :])
            xt_sb = consts.tile([128, KC, S], bf16, tag="xt")
            for half in range(2):
                for kc in range(KC):
                    nc.gpsimd.dma_start(
                        xt_sb[:, kc, half * 1024:(half + 1) * 1024],
                        xTd[kc * 128:(kc + 1) * 128,
                            half * 1024:(half + 1) * 1024])
            wv_sb = consts.tile([128, KC, HD], bf16, tag="wv")
            nc.gpsimd.dma_start(wv_sb, wvd.rearrange("(kc p) m -> p kc m", p=128))
            tri_sb = consts.tile([128, 128], bf16, tag="tri")
            nc.gpsimd.dma_start(tri_sb, trid[:, :])
            id_sb = consts.tile([128, 128], bf16, tag="id")
            nc.gpsimd.dma_start(id_sb, idd[:, :])
            wo_sb = consts.tile([128, 2, H], bf16, tag="wo")
            nc.gpsimd.dma_start(wo_sb, wod.rearrange("(kc p) m -> p kc m", p=128))

            qhat = [acts.tile([128, S], bf16, tag=f"qh{m}", name=f"qhat{m}")
                    for m in range(G)]
            khat = acts.tile([128, S], bf16, tag="khat")
            ktmp = acts.tile([128, S], bf16, tag="ktmp")
            v_sb = acts.tile([128, NST, HD + 1], bf16, tag="vsb")
            attn_n = [anp.tile([128, G * HD], bf16, tag=f"an{it}",
                               name=f"attn{it}")
                      for it in range(NST)]
            expT = [ep.tile([128, S], bf16, tag=f"e{jt}", name=f"expT{jt}")
                    for jt in range(NST)]
            aT = [acts.tile([128, S], bf16, tag=f"aT{c}", name=f"aTc{c}")
                  for c in range(2)]

            # attnV accumulators: slice `it` = bank[it//7][:, (it%7)*65 :+65]
            avb = [pav.tile([128, w], f32, tag=f"av{b}", name=f"avb{b}")
                   for b, w in ((0, 455), (1, 455), (2, 130))]

            def av_slice(it):
                b, o = it // 7, (it % 7) * 65
                return avb[b][:, o:o + 65]

            # ---- projection helpers (through the shared bank) ----
            def proj_cols(m, dst, c0, c1):
                ps = pR7.tile([128, 512], f32, tag="r7", name="psw")
                for kc in range(KC):
                    nc.tensor.matmul(
                        ps[:, 0:c1 - c0], wqk_sb[:, kc, m * 128:(m + 1) * 128],
                        xt_sb[:, kc, c0:c1],
                        start=(kc == 0), stop=(kc == KC - 1))
                nc.vector.tensor_mul(
                    dst[:, c0:c1], ps[:, 0:c1 - c0], cs_sb[:, c0:c1])

            def kfold_cols(c0, c1):
                psf = pR7.tile([128, 512], f32, tag="r7", name="psf")
                nc.tensor.matmul(psf[:, 0:c1 - c0], dj_sb, ktmp[:, c0:c1],
                                 start=True, stop=True)
                nc.vector.tensor_copy(khat[:, c0:c1], psf[:, 0:c1 - c0])

            def v_proj(st):
                psv = pR7.tile([128, HD], f32, tag="r7", name="psv")
                for kc in range(KC):
                    nc.tensor.matmul(
                        psv, xt_sb[:, kc, st * 128:(st + 1) * 128],
                        wv_sb[:, kc, :],
                        start=(kc == 0), stop=(kc == KC - 1))
                nc.vector.tensor_copy(v_sb[:, st, 0:HD], psv)

            # ---- scores + exp units through the pT pool ----
            def issue_units(h, jt, units):
                lo = jt * 128
                lhsT = khat[:, lo:lo + 128]
                for (a, b) in units:
                    base = (a // 1024) * 1024
                    T = pT.tile([128, 1024], f32, tag="T", name="pss")
                    c0 = (a // 512) * 512
                    while c0 < b:
                        s0, s1 = max(a, c0), min(b, c0 + 512)
                        nc.tensor.matmul(
                            T[:, s0 - base:s1 - base], lhsT,
                            qhat[h][:, s0:s1], start=True, stop=True)
                        c0 += 512
                    nc.scalar.activation(
                        expT[jt][:, a:b], T[:, a - base:b - base],
                        mybir.ActivationFunctionType.Exp, scale=scale)

            def issue_etri(jt):
                lo = jt * 128
                etri = etp.tile([128, 128], bf16, tag="et", name="etri")
                nc.gpsimd.tensor_mul(etri, expT[jt][:, lo:lo + 128], tri_sb)
                return etri

            def units_of(jt):
                return [(max(jt * 128, u), u + 1024) for u in range(0, S, 1024)
                        if u + 1024 > jt * 128]

            # ---- attnV + normalize for (h, jt) ----
            def attn_v(h, jt, etri, bank_first):
                for it in range(NST - 1, jt - 1, -1):
                    lhs = (etri if it == jt
                           else expT[jt][:, it * 128:(it + 1) * 128])
                    b = it // 7
                    first = jt == 0 and b not in bank_first
                    mm = nc.tensor.matmul(
                        av_slice(it), lhs, v_sb[:, jt, :],
                        start=first, stop=(it == jt),
                        skip_group_check=True)
                    if first:
                        bank_first[b] = mm
                    elif jt == 0:
                        add_dep_helper(mm.ins, bank_first[b].ins,
                                       sync=False,
                                       reason="bank clear first")
                pso = av_slice(jt)
                rc = rcp.tile([128, 1], f32, tag="rc", name="rc")
                nc.vector.reciprocal(rc, pso[:, HD:HD + 1])
                nc.vector.tensor_scalar_mul(
                    attn_n[jt][:, h * HD:(h + 1) * HD], pso[:, 0:HD], rc)

            # ---- o-projection pieces ----
            def transp(it):
                for c in range(2):
                    psx = pR7.tile([128, 128], bf16, tag="r7", name="pst")
                    nc.tensor.transpose(
                        psx, attn_n[it][:, c * 128:(c + 1) * 128], id_sb)
                    nc.vector.tensor_copy(
                        aT[c][:, it * 128:(it + 1) * 128], psx)

            def oproj_2hc(g, hc0):
                """o-proj of seq cols [512g:512g+512) for heads-chunks
                hc0, hc0+1 through one borrowed pT tile (2 psum regions)."""
                col = g * 512
                T2 = pT.tile([128, 1024], f32, tag="T", name="pso2")
                for i, hc in enumerate((hc0, hc0 + 1)):
                    for kc2 in range(2):
                        nc.tensor.matmul(
                            T2[:, i * 512:(i + 1) * 512],
                            wo_sb[:, kc2, hc * 128:(hc + 1) * 128],
                            aT[kc2][:, col:col + 512],
                            start=(kc2 == 0), stop=(kc2 == 1))
                for i, hc in enumerate((hc0, hc0 + 1)):
                    ot = otp.tile([128, 512], bf16, tag="ot", name="otst")
                    nc.vector.tensor_copy(ot, T2[:, i * 512:(i + 1) * 512])
                    nc.sync.dma_start(
                        oTd[hc * 128:(hc + 1) * 128, col:col + 512], ot)

            # ---- startup: k/q0 waves paced with the DMA column halves ----
            nc.vector.memset(v_sb[:, :, HD:HD + 1], 1.0)
            for c in range(2):
                proj_cols(G, ktmp, c * 512, (c + 1) * 512)
                kfold_cols(c * 512, (c + 1) * 512)
                proj_cols(0, qhat[0], c * 512, (c + 1) * 512)
            issue_units(0, 0, [(0, 512), (512, 1024)])
            et00 = issue_etri(0)
            v_proj(0)
            v_proj(1)
            for c in range(2, NSB):
                proj_cols(G, ktmp, c * 512, (c + 1) * 512)
                kfold_cols(c * 512, (c + 1) * 512)
                proj_cols(0, qhat[0], c * 512, (c + 1) * 512)
            issue_units(0, 0, [(1024, 2048)])
            etris = {(0, 0): et00}

            # ---- main attention loop, software-pipelined by one ----
            sched = [(h, jt) for h in range(G) for jt in range(NST)]
            bank_first = {}
            for idx, (h, jt) in enumerate(sched):
                # prefetch work (must precede the next-step issue: the next
                # head's scores depend on the last qhat prefetch piece)
                if h == 0 and jt + 2 < NST:
                    v_proj(jt + 2)
                if h < G - 1 and jt % 2 == 1:
                    c0 = (jt // 2) * 256
                    proj_cols(h + 1, qhat[h + 1], c0, c0 + 256)
                # issue next step's scores+exp ahead of this step's attnV
                if idx + 1 < len(sched):
                    h2, jt2 = sched[idx + 1]
                    issue_units(h2, jt2, units_of(jt2))
                    etris[(h2, jt2)] = issue_etri(jt2)
                if jt == 0:
                    bank_first = {}
                attn_v(h, jt, etris.pop((h, jt)), bank_first)
                if h == G - 1:
                    transp(jt)
                    if 8 <= jt < 14:
                        g, sub = (jt - 8) // 2, (jt - 8) % 2
                        oproj_2hc(g, 0 if sub == 0 else 4)
                        oproj_2hc(g, 2 if sub == 0 else 6)
            # tail: last o-proj group
            for hc0 in (0, 2, 4, 6):
                oproj_2hc(3, hc0)

    nc.finalize()
    return nc


def _host_inputs(hidden_states, position_ids, wq, wk, wv, wo):
    """Build the 8 per-core input maps."""
    def w2_of(w):
        # w: [64, H] rows of one head; returns sign-permuted rows
        w2 = np.empty_like(w)
        w2[:32] = -w[32:64]
        w2[32:] = w[:32]
        return w2

    dupJ = np.zeros((128, 128), np.float32)
    for p in range(128):
        dupJ[p, p % 64] = 1.0
        dupJ[p, p % 64 + 64] = 1.0
    dupJ = dupJ.astype(BF16)
    ident = np.eye(128, dtype=np.float32).astype(BF16)
    trimask = np.triu(np.ones((128, 128), np.float32)).astype(BF16)

    in_maps = []
    for core in range(N_CORES):
        b, kv = core // NKV, core % NKV
        xT = np.ascontiguousarray(hidden_states[b].T).astype(BF16)

        cols = []
        for i in range(G):
            h = kv * G + i
            wqh = wq[h * HD:(h + 1) * HD]
            cols.append(wqh.T)
            cols.append(w2_of(wqh).T)
        wkh = wk[kv * HD:(kv + 1) * HD]
        cols.append(wkh.T)
        cols.append(w2_of(wkh).T)
        wqkT = np.ascontiguousarray(np.concatenate(cols, axis=1)).astype(BF16)

        wvT = np.ascontiguousarray(wv[kv * HD:(kv + 1) * HD].T).astype(BF16)
        woT = np.ascontiguousarray(
            wo[:, kv * G * HD:(kv + 1) * G * HD].T).astype(BF16)

        inv = 1.0 / (THETA ** (np.arange(0, HD, 2, dtype=np.float32) / HD))
        freqs = position_ids[b].astype(np.float32)[:, None] * inv[None, :]
        emb = np.concatenate([freqs, freqs], axis=-1)       # [S, 64]
        cs = np.concatenate([np.cos(emb).T, np.sin(emb).T], axis=0)  # [128, S]
        cs = np.ascontiguousarray(cs).astype(BF16)

        in_maps.append({
            "xT": xT, "wqkT": wqkT, "wvT": wvT, "cs": cs, "woT": woT,
            "dupJ": dupJ, "ident": ident, "trimask": trimask,
        })
    return in_maps


_NC_CACHE = {}


def run_cores(in_maps, trace=False, trace_kwargs=None):
    from concourse.bass_utils import run_bass_kernel_spmd
    if "nc" not in _NC_CACHE:
        _NC_CACHE["nc"] = _build_nc()
    nc = _NC_CACHE["nc"]
    return run_bass_kernel_spmd(
        nc, in_maps, core_ids=list(range(N_CORES)),
        trace=trace, **(trace_kwargs or {}))


def kernel(hidden_states, attention_mask, position_ids, wq, wk, wv, wo):
    hidden_states = np.asarray(hidden_states, dtype=np.float32)
    position_ids = np.asarray(position_ids)
    wq = np.asarray(wq, dtype=np.float32)
    wk = np.asarray(wk, dtype=np.float32)
    wv = np.asarray(wv, dtype=np.float32)
    wo = np.asarray(wo, dtype=np.float32)

    in_maps = _host_inputs(hidden_states, position_ids, wq, wk, wv, wo)
    res = run_cores(in_maps)

    out = np.zeros((B, S, H), np.float32)
    for core in range(N_CORES):
        b = core // NKV
        out[b] += res.results[core]["oT"].T.astype(np.float32)
    return out


# revision 6
# speedup vs baseline: 1.0362x; 1.0362x over previous
"""Trainium2 Bass kernel for MimiAttention (GQA + RoPE + causal softmax).

Problem: B=2, S=2048, H=1024, NH=16 q-heads, NKV=4 kv-heads, HD=64.
Sharding: 8 cores = 2 (batch) x 4 (kv-group).  Each core computes one batch's
attention for one GQA group (4 q-heads sharing 1 kv head) and the partial
o-projection for those heads; the host sums the 4 partials per batch.

Pipeline (v2 — ACT-overlapped):
  * RoPE via the sign-permuted weight trick (W2 rows): per head the projection
    produces [q*cos ; q2*sin] on 128 partitions, khat = [k_rot;k_rot] via one
    fold matmul, so scores contract over 128 dims in one matmul per 512 cols.
  * Scores psum: 2x [128,1024] fp32 tiles (4 banks); exp runs in up-to-1024
    col batches (one ACT instruction per unit) to amortize the ACT init cost.
  * The jt loop is software-pipelined by one: scores+exp of step N+1 are
    issued before attnV of step N, so PE works while ACT runs exp.
  * attnV accumulates [i,65] slices in 3 psum banks (col 64 = softmax denom
    via a ones-column in v); normalize per (h, jt) on DVE.
  * o-projection is spread across h==3, jt>=8, borrowing the score psum pool
    slots (keeps both streams double-buffered); last 512 cols in the tail.
"""

import numpy as np
import ml_dtypes

B, S, H = 2, 2048, 1024
NH, NKV, HD = 16, 4, 64
G = NH // NKV            # 4 q-heads per kv head
THETA = 10000.0
N_CORES = 8

BF16 = ml_dtypes.bfloat16


def _build_nc():
    import concourse.mybir as mybir
    import concourse.tile as tile
    from concourse.tile import add_dep_helper
    from concourse import bacc

    f32 = mybir.dt.float32
    bf16 = mybir.dt.bfloat16

    nc = bacc.Bacc("TRN2", target_bir_lowering=False)

    xTd = nc.dram_tensor("xT", [H, S], bf16, kind="ExternalInput")
    wqkd = nc.dram_tensor("wqkT", [H, 640], bf16, kind="ExternalInput")
    wvd = nc.dram_tensor("wvT", [H, HD], bf16, kind="ExternalInput")
    csd = nc.dram_tensor("cs", [128, S], bf16, kind="ExternalInput")
    wod = nc.dram_tensor("woT", [G * HD, H], bf16, kind="ExternalInput")
    djd = nc.dram_tensor("dupJ", [128, 128], bf16, kind="ExternalInput")
    idd = nc.dram_tensor("ident", [128, 128], bf16, kind="ExternalInput")
    trid = nc.dram_tensor("trimask", [128, 128], bf16, kind="ExternalInput")
    oTd = nc.dram_tensor("oT", [H, S], bf16, kind="ExternalOutput")

    NSB = S // 512        # 4 chunks of 512
    NST = S // 128        # 16 tiles of 128
    KC = H // 128         # 8 contraction chunks
    scale = float(1.0 / np.sqrt(HD))

    with tile.TileContext(nc) as tc:
        import contextlib
        ctx = contextlib.ExitStack()
        with ctx:
            consts = ctx.enter_context(tc.tile_pool(name="consts", bufs=1))
            acts = ctx.enter_context(tc.tile_pool(name="acts", bufs=1))
            anp = ctx.enter_context(tc.tile_pool(name="attn", bufs=1))
            rcp = ctx.enter_context(tc.tile_pool(name="rcp", bufs=6))
            etp = ctx.enter_context(tc.tile_pool(name="etri", bufs=4))
            ep = ctx.enter_context(tc.tile_pool(name="exps", bufs=1))
            otp = ctx.enter_context(tc.tile_pool(name="ot", bufs=8))
            # PSUM: 2x [128,1024] score tiles (4 banks, also host o-proj
            # chunks at h==3), attnV accum (3 banks), 1 shared bank.
            pT = ctx.enter_context(
                tc.tile_pool(name="pT", bufs=2, space="PSUM"))
            pav = ctx.enter_context(
                tc.tile_pool(name="ps_av", bufs=1, space="PSUM"))
            pR7 = ctx.enter_context(
                tc.tile_pool(name="r7", bufs=1, space="PSUM"))

            # ---- input DMAs (issued from Pool seq; cheap dispatch).
            wqk_sb = consts.tile([128, KC, 640], bf16, tag="wqk")
            for kc in range(KC):
                nc.sync.dma_start(
                    wqk_sb[:, kc, :], wqkd[kc * 128:(kc + 1) * 128, :])
            cs_sb = consts.tile([128, S], bf16, tag="cs")
            nc.sync.dma_start(cs_sb, csd[:, :])
            dj_sb = consts.tile([128, 128], bf16, tag="dj")
            nc.sync.dma_start(dj_sb, djd[:, :])
            xt_sb = consts.tile([128, KC, S], bf16, tag="xt")
            for half in range(2):
                for kc in range(KC):
                    nc.sync.dma_start(
                        xt_sb[:, kc, half * 1024:(half + 1) * 1024],
                        xTd[kc * 128:(kc + 1) * 128,
                            half * 1024:(half + 1) * 1024])
            wv_sb = consts.tile([128, KC, HD], bf16, tag="wv")
            nc.sync.dma_start(wv_sb, wvd.rearrange("(kc p) m -> p kc m", p=128))
            tri_sb = consts.tile([128, 128], bf16, tag="tri")
            nc.sync.dma_start(tri_sb, trid[:, :])
            id_sb = consts.tile([128, 128], bf16, tag="id")
            nc.sync.dma_start(id_sb, idd[:, :])
            wo_sb = consts.tile([128, 2, H], bf16, tag="wo")
            nc.sync.dma_start(wo_sb, wod.rearrange("(kc p) m -> p kc m", p=128))

            qhat = [acts.tile([128, S], bf16, tag=f"qh{m}", name=f"qhat{m}")
                    for m in range(G)]
            khat = acts.tile([128, S], bf16, tag="khat")
            ktmp = acts.tile([128, S], bf16, tag="ktmp")
            v_sb = acts.tile([128, NST, HD + 1], bf16, tag="vsb")
            attn_n = [anp.tile([128, G * HD], bf16, tag=f"an{it}",
                               name=f"attn{it}")
                      for it in range(NST)]
            expT = [ep.tile([128, S], bf16, tag=f"e{jt}", name=f"expT{jt}")
                    for jt in range(NST)]
            aT = [acts.tile([128, S], bf16, tag=f"aT{c}", name=f"aTc{c}")
                  for c in range(2)]

            # attnV accumulators: slice `it` = bank[it//7][:, (it%7)*65 :+65]
            avb = [pav.tile([128, w], f32, tag=f"av{b}", name=f"avb{b}")
                   for b, w in ((0, 455), (1, 455), (2, 130))]

            def av_slice(it):
                b, o = it // 7, (it % 7) * 65
                return avb[b][:, o:o + 65]

            # ---- projection helpers (through the shared bank) ----
            def proj_cols(m, dst, c0, c1):
                ps = pR7.tile([128, 512], f32, tag="r7", name="psw")
                for kc in range(KC):
                    nc.tensor.matmul(
                        ps[:, 0:c1 - c0], wqk_sb[:, kc, m * 128:(m + 1) * 128],
                        xt_sb[:, kc, c0:c1],
                        start=(kc == 0), stop=(kc == KC - 1))
                nc.vector.tensor_mul(
                    dst[:, c0:c1], ps[:, 0:c1 - c0], cs_sb[:, c0:c1])

            def kfold_cols(c0, c1):
                psf = pR7.tile([128, 512], f32, tag="r7", name="psf")
                nc.tensor.matmul(psf[:, 0:c1 - c0], dj_sb, ktmp[:, c0:c1],
                                 start=True, stop=True)
                nc.vector.tensor_copy(khat[:, c0:c1], psf[:, 0:c1 - c0])

            def v_proj(st):
                psv = pR7.tile([128, HD], f32, tag="r7", name="psv")
                for kc in range(KC):
                    nc.tensor.matmul(
                        psv, xt_sb[:, kc, st * 128:(st + 1) * 128],
                        wv_sb[:, kc, :],
                        start=(kc == 0), stop=(kc == KC - 1))
                nc.vector.tensor_copy(v_sb[:, st, 0:HD], psv)

            # ---- scores + exp units through the pT pool ----
            def issue_units(h, jt, units):
                lo = jt * 128
                lhsT = khat[:, lo:lo + 128]
                for (a, b) in units:
                    base = (a // 1024) * 1024
                    T = pT.tile([128, 1024], f32, tag="T", name="pss")
                    c0 = (a // 512) * 512
                    while c0 < b:
                        s0, s1 = max(a, c0), min(b, c0 + 512)
                        nc.tensor.matmul(
                            T[:, s0 - base:s1 - base], lhsT,
                            qhat[h][:, s0:s1], start=True, stop=True)
                        c0 += 512
                    nc.scalar.activation(
                        expT[jt][:, a:b], T[:, a - base:b - base],
                        mybir.ActivationFunctionType.Exp, scale=scale)

            def issue_etri(jt):
                lo = jt * 128
                etri = etp.tile([128, 128], bf16, tag="et", name="etri")
                nc.gpsimd.tensor_mul(etri, expT[jt][:, lo:lo + 128], tri_sb)
                return etri

            def units_of(jt):
                return [(max(jt * 128, u), u + 1024) for u in range(0, S, 1024)
                        if u + 1024 > jt * 128]

            # ---- attnV + normalize for (h, jt) ----
            def attn_v(h, jt, etri, bank_first):
                for it in range(NST - 1, jt - 1, -1):
                    lhs = (etri if it == jt
                           else expT[jt][:, it * 128:(it + 1) * 128])
                    b = it // 7
                    first = jt == 0 and b not in bank_first
                    mm = nc.tensor.matmul(
                        av_slice(it), lhs, v_sb[:, jt, :],
                        start=first, stop=(it == jt),
                        skip_group_check=True)
                    if first:
                        bank_first[b] = mm
                    elif jt == 0:
                        add_dep_helper(mm.ins, bank_first[b].ins,
                                       sync=False,
                                       reason="bank clear first")
                pso = av_slice(jt)
                rc = rcp.tile([128, 1], f32, tag="rc", name="rc")
                nc.vector.reciprocal(rc, pso[:, HD:HD + 1])
                nc.vector.tensor_scalar_mul(
                    attn_n[jt][:, h * HD:(h + 1) * HD], pso[:, 0:HD], rc)

            # ---- o-projection pieces ----
            def transp(it):
                for c in range(2):
                    psx = pR7.tile([128, 128], bf16, tag="r7", name="pst")
                    nc.tensor.transpose(
                        psx, attn_n[it][:, c * 128:(c + 1) * 128], id_sb)
                    nc.vector.tensor_copy(
                        aT[c][:, it * 128:(it + 1) * 128], psx)

            def oproj_2hc(g, hc0):
                """o-proj of seq cols [512g:512g+512) for heads-chunks
                hc0, hc0+1 through one borrowed pT tile (2 psum regions)."""
                col = g * 512
                T2 = pT.tile([128, 1024], f32, tag="T", name="pso2")
                for i, hc in enumerate((hc0, hc0 + 1)):
                    for kc2 in range(2):
                        nc.tensor.matmul(
                            T2[:, i * 512:(i + 1) * 512],
                            wo_sb[:, kc2, hc * 128:(hc + 1) * 128],
                            aT[kc2][:, col:col + 512],
                            start=(kc2 == 0), stop=(kc2 == 1))
                for i, hc in enumerate((hc0, hc0 + 1)):
                    ot = otp.tile([128, 512], bf16, tag="ot", name="otst")
                    nc.vector.tensor_copy(ot, T2[:, i * 512:(i + 1) * 512])
                    nc.sync.dma_start(
                        oTd[hc * 128:(hc + 1) * 128, col:col + 512], ot)

            # ---- startup: k/q0 waves paced with the DMA column halves ----
            nc.vector.memset(v_sb[:, :, HD:HD + 1], 1.0)
            for c in range(2):
                proj_cols(G, ktmp, c * 512, (c + 1) * 512)
                kfold_cols(c * 512, (c + 1) * 512)
                proj_cols(0, qhat[0], c * 512, (c + 1) * 512)
            issue_units(0, 0, [(0, 512), (512, 1024)])
            et00 = issue_etri(0)
            v_proj(0)
            v_proj(1)
            for c in range(2, NSB):
                proj_cols(G, ktmp, c * 512, (c + 1) * 512)
                kfold_cols(c * 512, (c + 1) * 512)
                proj_cols(0, qhat[0], c * 512, (c + 1) * 512)
            issue_units(0, 0, [(1024, 2048)])
            etris = {(0, 0): et00}

            # ---- main attention loop, software-pipelined by one ----
            sched = [(h, jt) for h in range(G) for jt in range(NST)]
            bank_first = {}
            for idx, (h, jt) in enumerate(sched):
                # prefetch work (must precede the next-step issue: the next
                # head's scores depend on the last qhat prefetch piece)
                if h == 0 and jt + 2 < NST:
                    v_proj(jt + 2)
                if h < G - 1 and jt % 2 == 1:
                    c0 = (jt // 2) * 256
                    proj_cols(h + 1, qhat[h + 1], c0, c0 + 256)
                # issue next step's scores+exp ahead of this step's attnV
                if idx + 1 < len(sched):
                    h2, jt2 = sched[idx + 1]
                    issue_units(h2, jt2, units_of(jt2))
                    etris[(h2, jt2)] = issue_etri(jt2)
                if jt == 0:
                    bank_first = {}
                attn_v(h, jt, etris.pop((h, jt)), bank_first)
                if h == G - 1:
                    transp(jt)
                    if 8 <= jt < 14:
                        g, sub = (jt - 8) // 2, (jt - 8) % 2
                        oproj_2hc(g, 0 if sub == 0 else 4)
                        oproj_2hc(g, 2 if sub == 0 else 6)
            # tail: last o-proj group
            for hc0 in (0, 2, 4, 6):
                oproj_2hc(3, hc0)

    nc.finalize()
    return nc


def _host_inputs(hidden_states, position_ids, wq, wk, wv, wo):
    """Build the 8 per-core input maps."""
    def w2_of(w):
        # w: [64, H] rows of one head; returns sign-permuted rows
        w2 = np.empty_like(w)
        w2[:32] = -w[32:64]
        w2[32:] = w[:32]
        return w2

    dupJ = np.zeros((128, 128), np.float32)
    for p in range(128):
        dupJ[p, p % 64] = 1.0
        dupJ[p, p % 64 + 64] = 1.0
    dupJ = dupJ.astype(BF16)
    ident = np.eye(128, dtype=np.float32).astype(BF16)
    trimask = np.triu(np.ones((128, 128), np.float32)).astype(BF16)

    in_maps = []
    for core in range(N_CORES):
        b, kv = core // NKV, core % NKV
        xT = np.ascontiguousarray(hidden_states[b].T).astype(BF16)

        cols = []
        for i in range(G):
            h = kv * G + i
            wqh = wq[h * HD:(h + 1) * HD]
            cols.append(wqh.T)
            cols.append(w2_of(wqh).T)
        wkh = wk[kv * HD:(kv + 1) * HD]
        cols.append(wkh.T)
        cols.append(w2_of(wkh).T)
        wqkT = np.ascontiguousarray(np.concatenate(cols, axis=1)).astype(BF16)

        wvT = np.ascontiguousarray(wv[kv * HD:(kv + 1) * HD].T).astype(BF16)
        woT = np.ascontiguousarray(
            wo[:, kv * G * HD:(kv + 1) * G * HD].T).astype(BF16)

        inv = 1.0 / (THETA ** (np.arange(0, HD, 2, dtype=np.float32) / HD))
        freqs = position_ids[b].astype(np.float32)[:, None] * inv[None, :]
        emb = np.concatenate([freqs, freqs], axis=-1)       # [S, 64]
        cs = np.concatenate([np.cos(emb).T, np.sin(emb).T], axis=0)  # [128, S]
        cs = np.ascontiguousarray(cs).astype(BF16)

        in_maps.append({
            "xT": xT, "wqkT": wqkT, "wvT": wvT, "cs": cs, "woT": woT,
            "dupJ": dupJ, "ident": ident, "trimask": trimask,
        })
    return in_maps


_NC_CACHE = {}


def run_cores(in_maps, trace=False, trace_kwargs=None):
    from concourse.bass_utils import run_bass_kernel_spmd
    if "nc" not in _NC_CACHE:
        _NC_CACHE["nc"] = _build_nc()
    nc = _NC_CACHE["nc"]
    return run_bass_kernel_spmd(
        nc, in_maps, core_ids=list(range(N_CORES)),
        trace=trace, **(trace_kwargs or {}))


def kernel(hidden_states, attention_mask, position_ids, wq, wk, wv, wo):
    hidden_states = np.asarray(hidden_states, dtype=np.float32)
    position_ids = np.asarray(position_ids)
    wq = np.asarray(wq, dtype=np.float32)
    wk = np.asarray(wk, dtype=np.float32)
    wv = np.asarray(wv, dtype=np.float32)
    wo = np.asarray(wo, dtype=np.float32)

    in_maps = _host_inputs(hidden_states, position_ids, wq, wk, wv, wo)
    res = run_cores(in_maps)

    out = np.zeros((B, S, H), np.float32)
    for core in range(N_CORES):
        b = core // NKV
        out[b] += res.results[core]["oT"].T.astype(np.float32)
    return out


# revision 7
# speedup vs baseline: 1.0737x; 1.0362x over previous
"""Trainium2 Bass kernel for MimiAttention (GQA + RoPE + causal softmax).

Problem: B=2, S=2048, H=1024, NH=16 q-heads, NKV=4 kv-heads, HD=64.
Sharding: 8 cores = 2 (batch) x 4 (kv-group).  Each core computes one batch's
attention for one GQA group (4 q-heads sharing 1 kv head) and the partial
o-projection for those heads; the host sums the 4 partials per batch.

Pipeline (v2 — ACT-overlapped):
  * RoPE via the sign-permuted weight trick (W2 rows): per head the projection
    produces [q*cos ; q2*sin] on 128 partitions, khat = [k_rot;k_rot] via one
    fold matmul, so scores contract over 128 dims in one matmul per 512 cols.
  * Scores psum: 2x [128,1024] fp32 tiles (4 banks); exp runs in up-to-1024
    col batches (one ACT instruction per unit) to amortize the ACT init cost.
  * The jt loop is software-pipelined by one: scores+exp of step N+1 are
    issued before attnV of step N, so PE works while ACT runs exp.
  * attnV accumulates [i,65] slices in 3 psum banks (col 64 = softmax denom
    via a ones-column in v); normalize per (h, jt) on DVE.
  * o-projection is spread across h==3, jt>=8, borrowing the score psum pool
    slots (keeps both streams double-buffered); last 512 cols in the tail.
"""

import numpy as np
import ml_dtypes

B, S, H = 2, 2048, 1024
NH, NKV, HD = 16, 4, 64
G = NH // NKV            # 4 q-heads per kv head
THETA = 10000.0
N_CORES = 8

BF16 = ml_dtypes.bfloat16


def _build_nc():
    import concourse.mybir as mybir
    import concourse.tile as tile
    from concourse.tile import add_dep_helper
    from concourse import bacc

    f32 = mybir.dt.float32
    bf16 = mybir.dt.bfloat16

    nc = bacc.Bacc("TRN2", target_bir_lowering=False)

    xTd = nc.dram_tensor("xT", [H, S], bf16, kind="ExternalInput")
    wqkd = nc.dram_tensor("wqkT", [H, 640], bf16, kind="ExternalInput")
    wvd = nc.dram_tensor("wvT", [H, HD], bf16, kind="ExternalInput")
    csd = nc.dram_tensor("cs", [128, S], bf16, kind="ExternalInput")
    wod = nc.dram_tensor("woT", [G * HD, H], bf16, kind="ExternalInput")
    djd = nc.dram_tensor("dupJ", [128, 128], bf16, kind="ExternalInput")
    idd = nc.dram_tensor("ident", [128, 128], bf16, kind="ExternalInput")
    trid = nc.dram_tensor("trimask", [128, 128], bf16, kind="ExternalInput")
    oTd = nc.dram_tensor("oT", [H, S], bf16, kind="ExternalOutput")

    NSB = S // 512        # 4 chunks of 512
    NST = S // 128        # 16 tiles of 128
    KC = H // 128         # 8 contraction chunks
    scale = float(1.0 / np.sqrt(HD))

    with tile.TileContext(nc) as tc:
        import contextlib
        ctx = contextlib.ExitStack()
        with ctx:
            consts = ctx.enter_context(tc.tile_pool(name="consts", bufs=1))
            acts = ctx.enter_context(tc.tile_pool(name="acts", bufs=1))
            anp = ctx.enter_context(tc.tile_pool(name="attn", bufs=1))
            rcp = ctx.enter_context(tc.tile_pool(name="rcp", bufs=6))
            etp = ctx.enter_context(tc.tile_pool(name="etri", bufs=4))
            ep = ctx.enter_context(tc.tile_pool(name="exps", bufs=1))
            otp = ctx.enter_context(tc.tile_pool(name="ot", bufs=8))
            # PSUM: 2x [128,1024] score tiles (4 banks, also host o-proj
            # chunks at h==3), attnV accum (3 banks), 1 shared bank.
            pT = ctx.enter_context(
                tc.tile_pool(name="pT", bufs=2, space="PSUM"))
            pav = ctx.enter_context(
                tc.tile_pool(name="ps_av", bufs=1, space="PSUM"))
            pR7 = ctx.enter_context(
                tc.tile_pool(name="r7", bufs=1, space="PSUM"))

            # ---- input DMAs (issued from Pool seq; cheap dispatch).
            wqk_sb = consts.tile([128, KC, 640], bf16, tag="wqk")
            for kc in range(KC):
                nc.sync.dma_start(
                    wqk_sb[:, kc, :], wqkd[kc * 128:(kc + 1) * 128, :])
            cs_sb = consts.tile([128, S], bf16, tag="cs")
            nc.sync.dma_start(cs_sb, csd[:, :])
            dj_sb = consts.tile([128, 128], bf16, tag="dj")
            nc.sync.dma_start(dj_sb, djd[:, :])
            xt_sb = [consts.tile([128, KC, 1024], bf16, tag=f"xt{half}",
                                 name=f"xt{half}")
                     for half in range(2)]
            for half in range(2):
                for kc in range(KC):
                    nc.sync.dma_start(
                        xt_sb[half][:, kc, :],
                        xTd[kc * 128:(kc + 1) * 128,
                            half * 1024:(half + 1) * 1024])
            wv_sb = consts.tile([128, KC, HD], bf16, tag="wv")
            nc.sync.dma_start(wv_sb, wvd.rearrange("(kc p) m -> p kc m", p=128))
            tri_sb = consts.tile([128, 128], bf16, tag="tri")
            nc.sync.dma_start(tri_sb, trid[:, :])
            id_sb = consts.tile([128, 128], bf16, tag="id")
            nc.sync.dma_start(id_sb, idd[:, :])
            wo_sb = consts.tile([128, 2, H], bf16, tag="wo")
            nc.sync.dma_start(wo_sb, wod.rearrange("(kc p) m -> p kc m", p=128))

            qhat = [acts.tile([128, S], bf16, tag=f"qh{m}", name=f"qhat{m}")
                    for m in range(G)]
            khat = acts.tile([128, S], bf16, tag="khat")
            ktmp = acts.tile([128, S], bf16, tag="ktmp")
            v_sb = acts.tile([128, NST, HD + 1], bf16, tag="vsb")
            attn_n = [anp.tile([128, G * HD], bf16, tag=f"an{it}",
                               name=f"attn{it}")
                      for it in range(NST)]
            expT = [ep.tile([128, S], bf16, tag=f"e{jt}", name=f"expT{jt}")
                    for jt in range(NST)]
            aT = [acts.tile([128, S], bf16, tag=f"aT{c}", name=f"aTc{c}")
                  for c in range(2)]

            # attnV accumulators: slice `it` = bank[it//7][:, (it%7)*65 :+65]
            avb = [pav.tile([128, w], f32, tag=f"av{b}", name=f"avb{b}")
                   for b, w in ((0, 455), (1, 455), (2, 130))]

            def av_slice(it):
                b, o = it // 7, (it % 7) * 65
                return avb[b][:, o:o + 65]

            # ---- projection helpers ----
            def xt_cols(kc, c0, c1):
                half = c0 // 1024
                return xt_sb[half][:, kc, c0 - half * 1024:c1 - half * 1024]

            def proj_cols(m, dst, c0, c1, pool=None):
                pool = pool or pR7
                ps = pool.tile([128, 512], f32, tag=pool is pR7 and "r7" or "T",
                               name="psw")
                for kc in range(KC):
                    nc.tensor.matmul(
                        ps[:, 0:c1 - c0], wqk_sb[:, kc, m * 128:(m + 1) * 128],
                        xt_cols(kc, c0, c1),
                        start=(kc == 0), stop=(kc == KC - 1))
                nc.vector.tensor_mul(
                    dst[:, c0:c1], ps[:, 0:c1 - c0], cs_sb[:, c0:c1])

            def kfold_cols(c0, c1, pool=None):
                pool = pool or pR7
                psf = pool.tile([128, 512], f32,
                                tag=pool is pR7 and "r7" or "T", name="psf")
                nc.tensor.matmul(psf[:, 0:c1 - c0], dj_sb, ktmp[:, c0:c1],
                                 start=True, stop=True)
                nc.vector.tensor_copy(khat[:, c0:c1], psf[:, 0:c1 - c0])

            def v_proj(st):
                psv = pR7.tile([128, HD], f32, tag="r7", name="psv")
                for kc in range(KC):
                    nc.tensor.matmul(
                        psv, xt_cols(kc, st * 128, (st + 1) * 128),
                        wv_sb[:, kc, :],
                        start=(kc == 0), stop=(kc == KC - 1))
                nc.vector.tensor_copy(v_sb[:, st, 0:HD], psv)

            # ---- scores + exp units through the pT pool ----
            def issue_units(h, jt, units):
                lo = jt * 128
                lhsT = khat[:, lo:lo + 128]
                for (a, b) in units:
                    base = (a // 1024) * 1024
                    T = pT.tile([128, 1024], f32, tag="T", name="pss")
                    c0 = (a // 512) * 512
                    while c0 < b:
                        s0, s1 = max(a, c0), min(b, c0 + 512)
                        nc.tensor.matmul(
                            T[:, s0 - base:s1 - base], lhsT,
                            qhat[h][:, s0:s1], start=True, stop=True)
                        c0 += 512
                    nc.scalar.activation(
                        expT[jt][:, a:b], T[:, a - base:b - base],
                        mybir.ActivationFunctionType.Exp, scale=scale)

            def issue_etri(jt):
                lo = jt * 128
                etri = etp.tile([128, 128], bf16, tag="et", name="etri")
                nc.gpsimd.tensor_mul(etri, expT[jt][:, lo:lo + 128], tri_sb)
                return etri

            def units_of(jt):
                return [(max(jt * 128, u), u + 1024) for u in range(0, S, 1024)
                        if u + 1024 > jt * 128]

            # ---- attnV + normalize for (h, jt) ----
            def attn_v(h, jt, etri, bank_first):
                for it in range(NST - 1, jt - 1, -1):
                    lhs = (etri if it == jt
                           else expT[jt][:, it * 128:(it + 1) * 128])
                    b = it // 7
                    first = jt == 0 and b not in bank_first
                    mm = nc.tensor.matmul(
                        av_slice(it), lhs, v_sb[:, jt, :],
                        start=first, stop=(it == jt),
                        skip_group_check=True)
                    if first:
                        bank_first[b] = mm
                    elif jt == 0:
                        add_dep_helper(mm.ins, bank_first[b].ins,
                                       sync=False,
                                       reason="bank clear first")
                pso = av_slice(jt)
                rc = rcp.tile([128, 1], f32, tag="rc", name="rc")
                nc.vector.reciprocal(rc, pso[:, HD:HD + 1])
                nc.vector.tensor_scalar_mul(
                    attn_n[jt][:, h * HD:(h + 1) * HD], pso[:, 0:HD], rc)

            # ---- o-projection pieces ----
            def transp(it):
                for c in range(2):
                    psx = pR7.tile([128, 128], bf16, tag="r7", name="pst")
                    nc.tensor.transpose(
                        psx, attn_n[it][:, c * 128:(c + 1) * 128], id_sb)
                    nc.vector.tensor_copy(
                        aT[c][:, it * 128:(it + 1) * 128], psx)

            def oproj_2hc(g, hc0):
                """o-proj of seq cols [512g:512g+512) for heads-chunks
                hc0, hc0+1 through one borrowed pT tile (2 psum regions)."""
                col = g * 512
                T2 = pT.tile([128, 1024], f32, tag="T", name="pso2")
                for i, hc in enumerate((hc0, hc0 + 1)):
                    for kc2 in range(2):
                        nc.tensor.matmul(
                            T2[:, i * 512:(i + 1) * 512],
                            wo_sb[:, kc2, hc * 128:(hc + 1) * 128],
                            aT[kc2][:, col:col + 512],
                            start=(kc2 == 0), stop=(kc2 == 1))
                for i, hc in enumerate((hc0, hc0 + 1)):
                    ot = otp.tile([128, 512], bf16, tag="ot", name="otst")
                    nc.vector.tensor_copy(ot, T2[:, i * 512:(i + 1) * 512])
                    nc.sync.dma_start(
                        oTd[hc * 128:(hc + 1) * 128, col:col + 512], ot)

            # ---- startup: k/q0 waves paced with the DMA column halves ----
            nc.vector.memset(v_sb[:, :, HD:HD + 1], 1.0)
            for c in range(2):
                proj_cols(G, ktmp, c * 512, (c + 1) * 512, pool=pT)
                kfold_cols(c * 512, (c + 1) * 512, pool=pT)
                proj_cols(0, qhat[0], c * 512, (c + 1) * 512, pool=pT)
            issue_units(0, 0, [(0, 512), (512, 1024)])
            et00 = issue_etri(0)
            v_proj(0)
            v_proj(1)
            for c in range(2, NSB):
                proj_cols(G, ktmp, c * 512, (c + 1) * 512, pool=pT)
                kfold_cols(c * 512, (c + 1) * 512, pool=pT)
                proj_cols(0, qhat[0], c * 512, (c + 1) * 512, pool=pT)
            issue_units(0, 0, [(1024, 2048)])
            etris = {(0, 0): et00}

            # ---- main attention loop, software-pipelined by one ----
            sched = [(h, jt) for h in range(G) for jt in range(NST)]
            bank_first = {}
            for idx, (h, jt) in enumerate(sched):
                # prefetch work (must precede the next-step issue: the next
                # head's scores depend on the last qhat prefetch piece)
                if h == 0 and jt + 2 < NST:
                    v_proj(jt + 2)
                if h < G - 1 and jt in (1, 3, 5, 7):
                    c0 = (jt // 2) * 512
                    proj_cols(h + 1, qhat[h + 1], c0, c0 + 512)
                # issue next step's scores+exp ahead of this step's attnV
                if idx + 1 < len(sched):
                    h2, jt2 = sched[idx + 1]
                    issue_units(h2, jt2, units_of(jt2))
                    etris[(h2, jt2)] = issue_etri(jt2)
                if jt == 0:
                    bank_first = {}
                attn_v(h, jt, etris.pop((h, jt)), bank_first)
                if h == G - 1:
                    transp(jt)
                    if 8 <= jt < 14:
                        g, sub = (jt - 8) // 2, (jt - 8) % 2
                        oproj_2hc(g, 0 if sub == 0 else 4)
                        oproj_2hc(g, 2 if sub == 0 else 6)
            # tail: last o-proj group
            for hc0 in (0, 2, 4, 6):
                oproj_2hc(3, hc0)

    nc.finalize()
    return nc


def _host_inputs(hidden_states, position_ids, wq, wk, wv, wo):
    """Build the 8 per-core input maps."""
    def w2_of(w):
        # w: [64, H] rows of one head; returns sign-permuted rows
        w2 = np.empty_like(w)
        w2[:32] = -w[32:64]
        w2[32:] = w[:32]
        return w2

    dupJ = np.zeros((128, 128), np.float32)
    for p in range(128):
        dupJ[p, p % 64] = 1.0
        dupJ[p, p % 64 + 64] = 1.0
    dupJ = dupJ.astype(BF16)
    ident = np.eye(128, dtype=np.float32).astype(BF16)
    trimask = np.triu(np.ones((128, 128), np.float32)).astype(BF16)

    in_maps = []
    for core in range(N_CORES):
        b, kv = core // NKV, core % NKV
        xT = np.ascontiguousarray(hidden_states[b].T).astype(BF16)

        cols = []
        for i in range(G):
            h = kv * G + i
            wqh = wq[h * HD:(h + 1) * HD]
            cols.append(wqh.T)
            cols.append(w2_of(wqh).T)
        wkh = wk[kv * HD:(kv + 1) * HD]
        cols.append(wkh.T)
        cols.append(w2_of(wkh).T)
        wqkT = np.ascontiguousarray(np.concatenate(cols, axis=1)).astype(BF16)

        wvT = np.ascontiguousarray(wv[kv * HD:(kv + 1) * HD].T).astype(BF16)
        woT = np.ascontiguousarray(
            wo[:, kv * G * HD:(kv + 1) * G * HD].T).astype(BF16)

        inv = 1.0 / (THETA ** (np.arange(0, HD, 2, dtype=np.float32) / HD))
        freqs = position_ids[b].astype(np.float32)[:, None] * inv[None, :]
        emb = np.concatenate([freqs, freqs], axis=-1)       # [S, 64]
        cs = np.concatenate([np.cos(emb).T, np.sin(emb).T], axis=0)  # [128, S]
        cs = np.ascontiguousarray(cs).astype(BF16)

        in_maps.append({
            "xT": xT, "wqkT": wqkT, "wvT": wvT, "cs": cs, "woT": woT,
            "dupJ": dupJ, "ident": ident, "trimask": trimask,
        })
    return in_maps


_NC_CACHE = {}


def run_cores(in_maps, trace=False, trace_kwargs=None):
    from concourse.bass_utils import run_bass_kernel_spmd
    if "nc" not in _NC_CACHE:
        _NC_CACHE["nc"] = _build_nc()
    nc = _NC_CACHE["nc"]
    return run_bass_kernel_spmd(
        nc, in_maps, core_ids=list(range(N_CORES)),
        trace=trace, **(trace_kwargs or {}))


def kernel(hidden_states, attention_mask, position_ids, wq, wk, wv, wo):
    hidden_states = np.asarray(hidden_states, dtype=np.float32)
    position_ids = np.asarray(position_ids)
    wq = np.asarray(wq, dtype=np.float32)
    wk = np.asarray(wk, dtype=np.float32)
    wv = np.asarray(wv, dtype=np.float32)
    wo = np.asarray(wo, dtype=np.float32)

    in_maps = _host_inputs(hidden_states, position_ids, wq, wk, wv, wo)
    res = run_cores(in_maps)

    out = np.zeros((B, S, H), np.float32)
    for core in range(N_CORES):
        b = core // NKV
        out[b] += res.results[core]["oT"].T.astype(np.float32)
    return out
